# revision 27
# baseline (speedup 1.0000x reference)
"""Multi-head causal attention (B=4, T=2048, C=1024, H=16, D=64) on 8 trn2 cores.

Sharding: tensor-parallel over heads within batch core-pairs.
  core c -> batch b = c//2, heads hoff..hoff+7 where hoff = (c%2)*8.
Each core:
  - projects Q^T/K^T (head-pairs packed to 128 partitions) and V (head-quads
    packed, stride-65 layout with a ones column folded in for free softmax sums)
  - causal attention per head in S^T = [j, i] orientation, exp without
    max-subtraction (scores are ~N(0, 0.25^2), safe), fp32r matmuls throughout
  - output projection to partial y^T [1024 c', 2048 t] (+ bo/2)
  - pairwise ReduceScatter (4 t-slabs) sums partner partials; core even keeps
    c' 0:512, odd keeps c' 512:1024.
Host reassembles the [B, T, C] output by transposing/concatenating slabs.
"""

import numpy as np
import ml_dtypes

import concourse.bass as bass
import concourse.mybir as mybir
from concourse import bacc
from concourse.tile import TileContext
from concourse.bass_utils import run_bass_kernel_spmd

F32 = mybir.dt.float32
F32R = mybir.dt.float32r
BF16 = mybir.dt.bfloat16
NPBF16 = ml_dtypes.bfloat16

B, T, C = 4, 2048, 1024
H, D = 16, 64
HC = 8           # heads per core
NPAIR = HC // 2  # head pairs (QK packing)
CCn = C // 128   # 8 contraction chunks
TTn = T // 512   # 4 query tiles of 512
JCn = T // 128   # 16 key chunks of 128
N_CORES = 8
RG = [[0, 1], [2, 3], [4, 5], [6, 7]]


def build_nc(with_rs: bool = True):
    nc = bacc.Bacc(None, target_bir_lowering=False)

    xT = nc.declare_dram_parameter("xT", [C, T], F32R, isOutput=False)
    wq = nc.declare_dram_parameter("wq", [C, 512], F32R, isOutput=False)
    wk = nc.declare_dram_parameter("wk", [C, 512], F32R, isOutput=False)
    wv = nc.declare_dram_parameter("wv", [C, 512], F32R, isOutput=False)
    wot = nc.declare_dram_parameter("wot", [512, C], BF16, isOutput=False)
    bo2 = nc.declare_dram_parameter("bo2", [128, 8], F32, isOutput=False)
    y = nc.declare_dram_parameter("y", [TTn, 512, 512], BF16, isOutput=True)
    consts = nc.declare_dram_parameter("consts", [128, 384], BF16, isOutput=False)

    with TileContext(nc) as tc:
        with (
            tc.tile_pool(name="persist", bufs=1) as persist,
            tc.tile_pool(name="psum", bufs=1, space="PSUM") as psum,
            tc.tile_pool(name="dram", bufs=1, space="DRAM") as dram,
        ):
            # ---- persistent tiles ----
            qt = [persist.tile([128, T], F32R, tag=f"qt{p}", name=f"qt{p}")
                  for p in range(NPAIR)]
            kt = [persist.tile([128, T], F32R, tag=f"kt{p}", name=f"kt{p}")
                  for p in range(NPAIR)]
            # V chunks: 8 heads * 65 cols (64 d + ones col for free softmax sums)
            v = [persist.tile([128, 65 * HC], BF16, tag=f"v{j}", name=f"v{j}")
                 for j in range(JCn)]
            ones8 = persist.tile([128, HC], BF16, tag="ones8")
            nc.vector.memset(ones8[:], 1.0)
            # bf16 causal-mask constants (host-provided): identity (moving),
            # trit = -30 strictly above diagonal, m30 = -30 everywhere
            cst = persist.tile([128, 384], BF16, tag="cst")
            nc.sync.dma_start(out=cst[:], in_=consts[:])
            identb = cst[:, 0:128]
            trit = cst[:, 128:256]
            m30 = cst[:, 256:384]
            ones1f = persist.tile([1, 64], F32, tag="ones1f")
            nc.vector.memset(ones1f[:], 1.0)
            ones1 = persist.tile([1, 64], F32R, tag="ones1")
            nc.vector.tensor_copy(ones1[:], ones1f[:])
            bo_sb = persist.tile([128, 8], F32, tag="bo_sb")
            nc.sync.dma_start(out=bo_sb[:], in_=bo2[:])
            pt_pool = persist

            y_parts = [dram.tile([1024, 512], BF16, name=f"y_part{t_}")
                       for t_ in range(TTn)]
            rs_outs = [dram.tile([512, 512], BF16, name=f"rs_out{t_}")
                       for t_ in range(TTn)]

            # ---- phase A: projections, streamed by t-slab ----
            with tc.tile_pool(name="xw", bufs=1) as xw:
                wqt = [xw.tile([128, 512], F32R, tag=f"wq{cc}", name=f"wq{cc}")
                       for cc in range(CCn)]
                wkt = [xw.tile([128, 512], F32R, tag=f"wk{cc}", name=f"wk{cc}")
                       for cc in range(CCn)]
                wvt = [xw.tile([128, 512], F32R, tag=f"wv{cc}", name=f"wv{cc}")
                       for cc in range(CCn)]
                def issue_xts(tt):
                    i0 = tt * 512
                    xts = [xw.tile([128, 512], F32R, tag=f"xt{cc}", bufs=2,
                                   name=f"xt{cc}_{tt}") for cc in range(CCn)]
                    for cc in range(CCn):
                        nc.sync.dma_start(
                            out=xts[cc][:], in_=xT[cc * 128:(cc + 1) * 128, i0:i0 + 512]
                        )
                    return xts

                xts0 = [xw.tile([128, 512], F32R, tag=f"xt{cc}", bufs=2,
                                name=f"xt{cc}_0") for cc in range(CCn)]
                def dma_wq(cc):
                    nc.sync.dma_start(out=wqt[cc][:], in_=wq[cc * 128:(cc + 1) * 128, :])
                def dma_x0(cc):
                    nc.sync.dma_start(out=xts0[cc][:], in_=xT[cc * 128:(cc + 1) * 128, 0:512])
                dma_wq(0); dma_x0(0)
                dma_wq(1); dma_x0(1); dma_wq(2); dma_x0(2)
                for cc in range(3, CCn):
                    dma_wq(cc)
                for cc in range(3, CCn):
                    dma_x0(cc)
                for cc in range(CCn):
                    nc.sync.dma_start(out=wkt[cc][:], in_=wk[cc * 128:(cc + 1) * 128, :])
                for cc in range(CCn):
                    nc.sync.dma_start(out=wvt[cc][:], in_=wv[cc * 128:(cc + 1) * 128, :])

                for tt in range(TTn):
                    i0 = tt * 512
                    xts = xts0 if tt == 0 else issue_xts(tt)
                    for wt, dst in ((wqt, qt), (wkt, kt)):
                        for p2 in range(NPAIR // 2):
                            ps = psum.tile([128, 1024], F32, tag="stps", bufs=2,
                                           name=f"aps{tt}{p2}")
                            for k in range(2):
                                p = 2 * p2 + k
                                for cc in range(CCn):
                                    nc.tensor.matmul(
                                        ps[:, k * 512:(k + 1) * 512],
                                        wt[cc][:, p * 128:(p + 1) * 128],
                                        xts[cc][:],
                                        start=(cc == 0), stop=(cc == CCn - 1),
                                        skip_group_check=True,
                                    )
                            for k in range(2):
                                nc.vector.tensor_copy(
                                    dst[2 * p2 + k][:, i0:i0 + 512],
                                    ps[:, k * 512:(k + 1) * 512],
                                )
                    for jc in range(4 * tt, 4 * tt + 4):
                        jl = jc * 128 - i0  # 0..383 within slab
                        ps = psum.tile([128, 512], F32, tag="ovps", bufs=2,
                                       name=f"vps{jc}")
                        for g in range(2):
                            for cc in range(CCn):
                                nc.tensor.matmul(
                                    ps[:, g * 256:(g + 1) * 256],
                                    xts[cc][:, jl:jl + 128],
                                    wvt[cc][:, g * 256:(g + 1) * 256],
                                    start=(cc == 0), stop=(cc == CCn - 1),
                                    skip_group_check=True,
                                )
                        dst_ap = v[jc][:].rearrange(
                            "p (h e) -> p h e", h=HC, e=65
                        )[:, :, 0:64]
                        nc.vector.tensor_copy(dst_ap, ps[:])
                        ones_ap = v[jc][:].rearrange(
                            "p (h e) -> p h e", h=HC, e=65
                        )[:, :, 64:65]
                        nc.vector.tensor_copy(ones_ap, ones8[:])

            # ---- phase B/C interleaved per tt ----
            with tc.tile_pool(name="bc_pool", bufs=1) as bcp:
                ot = [bcp.tile([128, T], BF16, tag=f"ot{p}", name=f"ot{p}")
                      for p in range(NPAIR)]
                wot_t = [bcp.tile([128, C], BF16, tag=f"wot{cl}", name=f"wot{cl}")
                         for cl in range(4)]
                for cl in range(4):
                    nc.sync.dma_start(
                        out=wot_t[cl][:], in_=wot[cl * 128:(cl + 1) * 128, :]
                    )

                held = None  # (ov, h, pt, kk, n_jc) AV group awaiting emission

                def emit_norm(pend):
                    nonlocal held
                    ov, p, e, i0 = pend
                    if held is not None and held[0] is ov:
                        emit_avs(held)
                        held = None
                    # rows 0:64 = unnormalized O^T, row 64 = softmax sum l
                    rl = bcp.tile([1, 512], F32, tag="rl", bufs=2)
                    nc.vector.reciprocal(rl[:], ov[64:65, :])
                    rlr = bcp.tile([1, 512], F32R, tag="rlr", bufs=2)
                    nc.vector.tensor_copy(rlr[:], rl[:])
                    bc = psum.tile([64, 512], F32, tag="yps", bufs=2)
                    nc.tensor.matmul(
                        bc[:], ones1[:], rlr[:], start=True, stop=True,
                        skip_group_check=True,
                    )
                    bc_sb = bcp.tile([64, 512], F32, tag="bc_sb", bufs=2)
                    nc.vector.tensor_copy(bc_sb[:], bc[:])
                    nc.vector.tensor_mul(
                        ot[p][e * 64:(e + 1) * 64, i0:i0 + 512],
                        ov[0:64, :], bc_sb[:],
                    )

                pending = None

                def emit_outproj_group(tt, cp):
                    i0 = tt * 512
                    yps = psum.tile([128, 512], F32, tag="yps", bufs=2,
                                    name=f"yps{tt}{cp}")
                    for cl in range(4):
                        nc.tensor.matmul(
                            yps[:],
                            wot_t[cl][:, cp * 128:(cp + 1) * 128],
                            ot[cl][:, i0:i0 + 512],
                            start=(cl == 0), stop=(cl == 3),
                            skip_group_check=True,
                        )
                    ysb = bcp.tile([128, 512], BF16, tag="ysb", bufs=4)
                    nc.vector.tensor_scalar_add(ysb[:], yps[:], bo_sb[:, cp:cp + 1])
                    nc.sync.dma_start(
                        out=y_parts[tt][cp * 128:(cp + 1) * 128, :], in_=ysb[:]
                    )

                def emit_rs(tt):
                    if with_rs:
                        nc.gpsimd.collective_compute(
                            "ReduceScatter",
                            mybir.AluOpType.add,
                            replica_groups=RG,
                            ins=[y_parts[tt][:]],
                            outs=[rs_outs[tt][:]],
                        )
                    else:
                        nc.sync.dma_start(out=y[tt], in_=y_parts[tt][0:512, :])

                def emit_avs(held):
                    ov, h, pt, kk, n_jc = held
                    for k in range(2):
                        jc, a = kk[k]
                        nc.tensor.matmul(
                            ov[:, a:512],
                            v[jc][:, h * 65:(h + 1) * 65],
                            pt[:, k * 512 + a:(k + 1) * 512],
                            start=(jc == 0), stop=(jc == n_jc - 1),
                            skip_group_check=True,
                        )

                for tt in range(TTn):
                    i0 = tt * 512
                    n_jc = 4 * (tt + 1)
                    for h in range(HC):
                        p, e = h // 2, h % 2
                        ov = psum.tile([65, 512], F32, tag="ovps", bufs=2,
                                      name=f"ov{tt}{h}")
                        for jc2 in range(n_jc // 2):
                            st = psum.tile([128, 1024], F32, tag="stps", bufs=2,
                                          name=f"st{tt}{h}{jc2}")
                            kk = []  # (jc, a) for the two chunks
                            for k in range(2):
                                jc = 2 * jc2 + k
                                kb = jc - 4 * tt  # band offset (>=0 within band)
                                a = min(kb * 128, 256) if kb >= 0 else 0
                                kk.append((jc, a))
                                nc.tensor.matmul(
                                    st[:, k * 512 + a:(k + 1) * 512],
                                    kt[p][e * 64:(e + 1) * 64,
                                          jc * 128:(jc + 1) * 128],
                                    qt[p][e * 64:(e + 1) * 64,
                                          i0 + a:i0 + 512],
                                    start=True, stop=True,
                                    skip_group_check=True,
                                )
                                kb_ = jc - 4 * tt
                                if kb_ >= 0:
                                    # causal mask: trit^T (-30 above diag)
                                    # onto the diagonal 128-block; fully
                                    # masked leading columns get m30
                                    blk = k * 512 + kb_ * 128
                                    nc.tensor.matmul(
                                        st[:, blk:blk + 128],
                                        trit, identb,
                                        start=False, stop=True,
                                        skip_group_check=True,
                                    )
                                    if a < kb_ * 128:
                                        nc.tensor.matmul(
                                            st[:, k * 512 + a:blk],
                                            m30, identb,
                                            start=False, stop=True,
                                            skip_group_check=True,
                                        )
                            # AV of the previously-held group (keeps PE fed
                            # while ACT works on this group's exp); crosses
                            # head boundaries so head h+1's QK never waits on
                            # head h's last exp chain.
                            if held is not None:
                                emit_avs(held)
                                held = None
                            if jc2 == 0 and tt >= 1 and 1 <= h <= 2:
                                # previous slab's outproj, two groups per head
                                # over the first heads so its RS fires early
                                # enough to overlap this slab's compute instead
                                # of stacking behind the next RS. norm(tt-1,h7)
                                # lands at (tt,h0,jc2==1), before these reads.
                                for g4 in range(4):
                                    emit_outproj_group(tt - 1, 4 * (h - 1) + g4)
                                if h == 2:
                                    emit_rs(tt - 1)
                            if jc2 == 1 and pending is not None:
                                emit_norm(pending)
                                pending = None
                            pt = pt_pool.tile([128, 1024], BF16, tag="pt", bufs=7,
                                              name=f"pt{tt}{h}{jc2}")
                            a0 = kk[0][1]
                            nc.scalar.activation(
                                pt[:, a0:1024], st[:, a0:1024],
                                mybir.ActivationFunctionType.Exp,
                            )
                            held = (ov, h, pt, kk, n_jc)
                        if pending is not None:  # tt0 heads have only 2 groups
                            emit_norm(pending)
                        pending = (ov, p, e, i0)
                    if tt == TTn - 1:
                        if pending is not None:
                            emit_norm(pending)
                            pending = None
                        for cp in range(8):
                            emit_outproj_group(tt, cp)
                        emit_rs(tt)
                        if with_rs:
                            # y <- rs_out copies at the very end: slab 0-2
                            # waits long satisfied; only y[3] waits the last
                            # collective, nothing queues behind them.
                            for t_ in range(TTn):
                                nc.sync.dma_start(out=y[t_], in_=rs_outs[t_][:])

    nc.compile()
    return nc


_NC_CACHE = {}


def _get_nc(with_rs: bool = True):
    key = bool(with_rs)
    if key not in _NC_CACHE:
        _NC_CACHE[key] = build_nc(with_rs)
    return _NC_CACHE[key]


def make_in_maps(x, Wq, Wk, Wv, Wo, bo):
    x = np.asarray(x, dtype=np.float32)
    Wq = np.asarray(Wq, dtype=np.float32)
    Wk = np.asarray(Wk, dtype=np.float32)
    Wv = np.asarray(Wv, dtype=np.float32)
    Wo = np.asarray(Wo, dtype=np.float32)
    bo = np.asarray(bo, dtype=np.float32)

    scale = np.float32(C) ** np.float32(-0.5)
    in_maps = []
    for c in range(N_CORES):
        b, hoff = c // 2, (c % 2) * HC
        heads = slice(hoff, hoff + HC)
        xT_c = np.ascontiguousarray(x[b].T)                      # [C, T]
        wq_c = np.ascontiguousarray(
            np.concatenate(list(Wq[heads] * scale), axis=1))     # [C, 512]
        wk_c = np.ascontiguousarray(np.concatenate(list(Wk[heads]), axis=1))
        wv_c = np.ascontiguousarray(np.concatenate(list(Wv[heads]), axis=1))
        wot_c = np.ascontiguousarray(
            Wo[:, hoff * D:(hoff + HC) * D].T.astype(NPBF16))    # [512, C]
        bo2_c = np.ascontiguousarray((bo / 2.0).reshape(8, 128).T)       # [128, 8]
        in_maps.append({
            "xT": xT_c, "wq": wq_c, "wk": wk_c, "wv": wv_c,
            "wot": wot_c, "bo2": bo2_c,
            "consts": np.ascontiguousarray(np.concatenate([
                np.eye(128, dtype=np.float32),
                np.triu(np.full((128, 128), -30.0, dtype=np.float32), k=1),
                np.full((128, 128), -30.0, dtype=np.float32),
            ], axis=1).astype(NPBF16)),
        })
    return in_maps


def kernel(x, Wq, Wk, Wv, Wo, bo):
    nc = _get_nc(with_rs=True)
    in_maps = make_in_maps(x, Wq, Wk, Wv, Wo, bo)
    # The axon-tunneled devices occasionally fail transiently
    # (NRT_EXEC_UNIT_UNRECOVERABLE / tunnel hangup); a retry recovers.
    last_err = None
    for _ in range(3):
        try:
            res = run_bass_kernel_spmd(nc, in_maps, list(range(N_CORES))).results
            break
        except Exception as e:  # noqa: BLE001
            last_err = e
            import time
            time.sleep(5)
    else:
        raise last_err

    out = np.empty((B, T, C), dtype=np.float32)
    for c in range(N_CORES):
        b, e = c // 2, c % 2
        yc = np.asarray(res[c]["y"], dtype=np.float32)  # [tt, c' slab, t]
        for tt in range(TTn):
            out[b, tt * 512:(tt + 1) * 512, e * 512:(e + 1) * 512] = yc[tt].T
    return out



# revision 29
# speedup vs baseline: 1.0406x; 1.0406x over previous
"""Multi-head causal attention (B=4, T=2048, C=1024, H=16, D=64) on 8 trn2 cores.

Sharding: tensor-parallel over heads within batch core-pairs.
  core c -> batch b = c//2, heads hoff..hoff+7 where hoff = (c%2)*8.
Each core:
  - projects Q^T/K^T (head-pairs packed to 128 partitions) and V (head-quads
    packed, stride-65 layout with a ones column folded in for free softmax sums)
  - causal attention per head in S^T = [j, i] orientation, exp without
    max-subtraction (scores are ~N(0, 0.25^2), safe), fp32r matmuls throughout
  - output projection to partial y^T [1024 c', 2048 t] (+ bo/2)
  - pairwise ReduceScatter (4 t-slabs) sums partner partials; core even keeps
    c' 0:512, odd keeps c' 512:1024.
Host reassembles the [B, T, C] output by transposing/concatenating slabs.
"""

import numpy as np
import ml_dtypes

import concourse.bass as bass
import concourse.mybir as mybir
from concourse import bacc
from concourse.tile import TileContext
from concourse.bass_utils import run_bass_kernel_spmd

F32 = mybir.dt.float32
F32R = mybir.dt.float32r
BF16 = mybir.dt.bfloat16
NPBF16 = ml_dtypes.bfloat16

B, T, C = 4, 2048, 1024
H, D = 16, 64
HC = 8           # heads per core
NPAIR = HC // 2  # head pairs (QK packing)
CCn = C // 128   # 8 contraction chunks
TTn = T // 512   # 4 query tiles of 512
JCn = T // 128   # 16 key chunks of 128
N_CORES = 8
RG = [[0, 1], [2, 3], [4, 5], [6, 7]]


def build_nc(with_rs: bool = True):
    nc = bacc.Bacc(None, target_bir_lowering=False)

    xT = nc.declare_dram_parameter("xT", [C, T], BF16, isOutput=False)
    wq = nc.declare_dram_parameter("wq", [C, 512], BF16, isOutput=False)
    wk = nc.declare_dram_parameter("wk", [C, 512], BF16, isOutput=False)
    wv = nc.declare_dram_parameter("wv", [C, 512], BF16, isOutput=False)
    wot = nc.declare_dram_parameter("wot", [512, C], BF16, isOutput=False)
    bo2 = nc.declare_dram_parameter("bo2", [128, 8], F32, isOutput=False)
    y = nc.declare_dram_parameter("y", [TTn, 512, 512], BF16, isOutput=True)
    consts = nc.declare_dram_parameter("consts", [128, 384], BF16, isOutput=False)

    with TileContext(nc) as tc:
        with (
            tc.tile_pool(name="persist", bufs=1) as persist,
            tc.tile_pool(name="psum", bufs=1, space="PSUM") as psum,
            tc.tile_pool(name="dram", bufs=1, space="DRAM") as dram,
        ):
            # ---- persistent tiles ----
            qt = [persist.tile([128, T], BF16, tag=f"qt{p}", name=f"qt{p}")
                  for p in range(NPAIR)]
            kt = [persist.tile([128, T], BF16, tag=f"kt{p}", name=f"kt{p}")
                  for p in range(NPAIR)]
            # V chunks: 8 heads * 65 cols (64 d + ones col for free softmax sums)
            v = [persist.tile([128, 65 * HC], BF16, tag=f"v{j}", name=f"v{j}")
                 for j in range(JCn)]
            ones8 = persist.tile([128, HC], BF16, tag="ones8")
            nc.vector.memset(ones8[:], 1.0)
            # bf16 causal-mask constants (host-provided): identity (moving),
            # trit = -30 strictly above diagonal, m30 = -30 everywhere
            cst = persist.tile([128, 384], BF16, tag="cst")
            nc.sync.dma_start(out=cst[:], in_=consts[:])
            identb = cst[:, 0:128]
            trit = cst[:, 128:256]
            m30 = cst[:, 256:384]
            ones1f = persist.tile([1, 64], F32, tag="ones1f")
            nc.vector.memset(ones1f[:], 1.0)
            ones1 = persist.tile([1, 64], F32R, tag="ones1")
            nc.vector.tensor_copy(ones1[:], ones1f[:])
            bo_sb = persist.tile([128, 8], F32, tag="bo_sb")
            nc.sync.dma_start(out=bo_sb[:], in_=bo2[:])
            pt_pool = persist

            y_parts = [dram.tile([1024, 512], BF16, name=f"y_part{t_}")
                       for t_ in range(TTn)]
            rs_outs = [dram.tile([512, 512], BF16, name=f"rs_out{t_}")
                       for t_ in range(TTn)]

            # ---- phase A: projections, streamed by t-slab ----
            with tc.tile_pool(name="xw", bufs=1) as xw:
                wqt = [xw.tile([128, 512], BF16, tag=f"wq{cc}", name=f"wq{cc}")
                       for cc in range(CCn)]
                wkt = [xw.tile([128, 512], BF16, tag=f"wk{cc}", name=f"wk{cc}")
                       for cc in range(CCn)]
                wvt = [xw.tile([128, 512], BF16, tag=f"wv{cc}", name=f"wv{cc}")
                       for cc in range(CCn)]
                def issue_xts(tt):
                    i0 = tt * 512
                    xts = [xw.tile([128, 512], BF16, tag=f"xt{cc}", bufs=2,
                                   name=f"xt{cc}_{tt}") for cc in range(CCn)]
                    for cc in range(CCn):
                        nc.sync.dma_start(
                            out=xts[cc][:], in_=xT[cc * 128:(cc + 1) * 128, i0:i0 + 512]
                        )
                    return xts

                xts0 = [xw.tile([128, 512], BF16, tag=f"xt{cc}", bufs=2,
                                name=f"xt{cc}_0") for cc in range(CCn)]
                def dma_wq(cc):
                    nc.sync.dma_start(out=wqt[cc][:], in_=wq[cc * 128:(cc + 1) * 128, :])
                def dma_x0(cc):
                    nc.sync.dma_start(out=xts0[cc][:], in_=xT[cc * 128:(cc + 1) * 128, 0:512])
                dma_wq(0); dma_x0(0)
                dma_wq(1); dma_x0(1); dma_wq(2); dma_x0(2)
                for cc in range(3, CCn):
                    dma_wq(cc)
                for cc in range(3, CCn):
                    dma_x0(cc)
                for cc in range(CCn):
                    nc.sync.dma_start(out=wkt[cc][:], in_=wk[cc * 128:(cc + 1) * 128, :])
                for cc in range(CCn):
                    nc.sync.dma_start(out=wvt[cc][:], in_=wv[cc * 128:(cc + 1) * 128, :])

                for tt in range(TTn):
                    i0 = tt * 512
                    xts = xts0 if tt == 0 else issue_xts(tt)
                    for wt, dst in ((wqt, qt), (wkt, kt)):
                        for p2 in range(NPAIR // 2):
                            ps = psum.tile([128, 1024], F32, tag="stps", bufs=2,
                                           name=f"aps{tt}{p2}")
                            for k in range(2):
                                p = 2 * p2 + k
                                for cc in range(CCn):
                                    nc.tensor.matmul(
                                        ps[:, k * 512:(k + 1) * 512],
                                        wt[cc][:, p * 128:(p + 1) * 128],
                                        xts[cc][:],
                                        start=(cc == 0), stop=(cc == CCn - 1),
                                        skip_group_check=True,
                                    )
                            for k in range(2):
                                nc.vector.tensor_copy(
                                    dst[2 * p2 + k][:, i0:i0 + 512],
                                    ps[:, k * 512:(k + 1) * 512],
                                )
                    for jc in range(4 * tt, 4 * tt + 4):
                        jl = jc * 128 - i0  # 0..383 within slab
                        ps = psum.tile([128, 512], F32, tag="ovps", bufs=2,
                                       name=f"vps{jc}")
                        for g in range(2):
                            for cc in range(CCn):
                                nc.tensor.matmul(
                                    ps[:, g * 256:(g + 1) * 256],
                                    xts[cc][:, jl:jl + 128],
                                    wvt[cc][:, g * 256:(g + 1) * 256],
                                    start=(cc == 0), stop=(cc == CCn - 1),
                                    skip_group_check=True,
                                )
                        dst_ap = v[jc][:].rearrange(
                            "p (h e) -> p h e", h=HC, e=65
                        )[:, :, 0:64]
                        nc.vector.tensor_copy(dst_ap, ps[:])
                        ones_ap = v[jc][:].rearrange(
                            "p (h e) -> p h e", h=HC, e=65
                        )[:, :, 64:65]
                        nc.vector.tensor_copy(ones_ap, ones8[:])

            # ---- phase B/C interleaved per tt ----
            with tc.tile_pool(name="bc_pool", bufs=1) as bcp:
                ot = [bcp.tile([128, T], BF16, tag=f"ot{p}", name=f"ot{p}")
                      for p in range(NPAIR)]
                wot_t = [bcp.tile([128, C], BF16, tag=f"wot{cl}", name=f"wot{cl}")
                         for cl in range(4)]
                for cl in range(4):
                    nc.sync.dma_start(
                        out=wot_t[cl][:], in_=wot[cl * 128:(cl + 1) * 128, :]
                    )

                held = None  # (ov, h, pt, kk, n_jc) AV group awaiting emission

                def emit_norm(pend):
                    nonlocal held
                    ov, p, e, i0 = pend
                    if held is not None and held[0] is ov:
                        emit_avs(held)
                        held = None
                    # rows 0:64 = unnormalized O^T, row 64 = softmax sum l
                    rlr = bcp.tile([1, 512], F32R, tag="rlr", bufs=2)
                    with nc.allow_low_precision(reason="f32r is fp32 bits"):
                        nc.vector.reciprocal(rlr[:], ov[64:65, :])
                    bc = psum.tile([64, 512], F32, tag="yps", bufs=2)
                    nc.tensor.matmul(
                        bc[:], ones1[:], rlr[:], start=True, stop=True,
                        skip_group_check=True,
                    )
                    bc_sb = bcp.tile([64, 512], F32, tag="bc_sb", bufs=2)
                    nc.vector.tensor_copy(bc_sb[:], bc[:])
                    nc.vector.tensor_mul(
                        ot[p][e * 64:(e + 1) * 64, i0:i0 + 512],
                        ov[0:64, :], bc_sb[:],
                    )

                pending = None

                def emit_outproj_group(tt, cp):
                    i0 = tt * 512
                    yps = psum.tile([128, 512], F32, tag="yps", bufs=2,
                                    name=f"yps{tt}{cp}")
                    for cl in range(4):
                        nc.tensor.matmul(
                            yps[:],
                            wot_t[cl][:, cp * 128:(cp + 1) * 128],
                            ot[cl][:, i0:i0 + 512],
                            start=(cl == 0), stop=(cl == 3),
                            skip_group_check=True,
                        )
                    ysb = bcp.tile([128, 512], BF16, tag="ysb", bufs=4)
                    nc.vector.tensor_scalar_add(ysb[:], yps[:], bo_sb[:, cp:cp + 1])
                    nc.sync.dma_start(
                        out=y_parts[tt][cp * 128:(cp + 1) * 128, :], in_=ysb[:]
                    )

                def emit_rs(tt):
                    if with_rs:
                        nc.gpsimd.collective_compute(
                            "ReduceScatter",
                            mybir.AluOpType.add,
                            replica_groups=RG,
                            ins=[y_parts[tt][:]],
                            outs=[rs_outs[tt][:]],
                        )
                    else:
                        nc.sync.dma_start(out=y[tt], in_=y_parts[tt][0:512, :])

                def emit_avs(held):
                    ov, h, pt, kk, n_jc = held
                    for k in range(2):
                        jc, a = kk[k]
                        nc.tensor.matmul(
                            ov[:, a:512],
                            v[jc][:, h * 65:(h + 1) * 65],
                            pt[:, k * 512 + a:(k + 1) * 512],
                            start=(jc == 0), stop=(jc == n_jc - 1),
                            skip_group_check=True,
                        )

                for tt in range(TTn):
                    i0 = tt * 512
                    n_jc = 4 * (tt + 1)
                    for h in range(HC):
                        p, e = h // 2, h % 2
                        ov = psum.tile([65, 512], F32, tag="ovps", bufs=2,
                                      name=f"ov{tt}{h}")
                        for jc2 in range(n_jc // 2):
                            st = psum.tile([128, 1024], F32, tag="stps", bufs=2,
                                          name=f"st{tt}{h}{jc2}")
                            kk = []  # (jc, a) for the two chunks
                            for k in range(2):
                                jc = 2 * jc2 + k
                                kb = jc - 4 * tt  # band offset (>=0 within band)
                                a = kb * 128 if kb >= 0 else 0
                                kk.append((jc, a))
                                nc.tensor.matmul(
                                    st[:, k * 512 + a:(k + 1) * 512],
                                    kt[p][e * 64:(e + 1) * 64,
                                          jc * 128:(jc + 1) * 128],
                                    qt[p][e * 64:(e + 1) * 64,
                                          i0 + a:i0 + 512],
                                    start=True, stop=True,
                                    skip_group_check=True,
                                )
                                kb_ = jc - 4 * tt
                                if kb_ >= 0:
                                    # causal mask: trit^T (-30 above diag)
                                    # onto the diagonal 128-block; fully
                                    # masked leading columns get m30
                                    blk = k * 512 + kb_ * 128
                                    nc.tensor.matmul(
                                        st[:, blk:blk + 128],
                                        trit, identb,
                                        start=False, stop=True,
                                        skip_group_check=True,
                                    )
                                    if a < kb_ * 128:
                                        nc.tensor.matmul(
                                            st[:, k * 512 + a:blk],
                                            m30, identb,
                                            start=False, stop=True,
                                            skip_group_check=True,
                                        )
                            # AV of the previously-held group (keeps PE fed
                            # while ACT works on this group's exp); crosses
                            # head boundaries so head h+1's QK never waits on
                            # head h's last exp chain.
                            if held is not None:
                                emit_avs(held)
                                held = None
                            if jc2 == 0 and tt >= 1 and 1 <= h <= 2:
                                # previous slab's outproj, two groups per head
                                # over the first heads so its RS fires early
                                # enough to overlap this slab's compute instead
                                # of stacking behind the next RS. norm(tt-1,h7)
                                # lands at (tt,h0,jc2==1), before these reads.
                                for g4 in range(4):
                                    emit_outproj_group(tt - 1, 4 * (h - 1) + g4)
                                if h == 2:
                                    emit_rs(tt - 1)
                            if jc2 == 1 and pending is not None:
                                emit_norm(pending)
                                pending = None
                            pt = pt_pool.tile([128, 1024], BF16, tag="pt", bufs=7,
                                              name=f"pt{tt}{h}{jc2}")
                            a0 = kk[0][1]
                            nc.scalar.activation(
                                pt[:, a0:1024], st[:, a0:1024],
                                mybir.ActivationFunctionType.Exp,
                            )
                            held = (ov, h, pt, kk, n_jc)
                        if pending is not None:  # tt0 heads have only 2 groups
                            emit_norm(pending)
                        pending = (ov, p, e, i0)
                    if tt == TTn - 1:
                        if pending is not None:
                            emit_norm(pending)
                            pending = None
                        for cp in range(8):
                            emit_outproj_group(tt, cp)
                        emit_rs(tt)
                        if with_rs:
                            # y <- rs_out copies at the very end: slab 0-2
                            # waits long satisfied; only y[3] waits the last
                            # collective, nothing queues behind them.
                            for t_ in range(TTn):
                                nc.sync.dma_start(out=y[t_], in_=rs_outs[t_][:])

    nc.compile()
    return nc


_NC_CACHE = {}


def _get_nc(with_rs: bool = True):
    key = bool(with_rs)
    if key not in _NC_CACHE:
        _NC_CACHE[key] = build_nc(with_rs)
    return _NC_CACHE[key]


def make_in_maps(x, Wq, Wk, Wv, Wo, bo):
    x = np.asarray(x, dtype=np.float32)
    Wq = np.asarray(Wq, dtype=np.float32)
    Wk = np.asarray(Wk, dtype=np.float32)
    Wv = np.asarray(Wv, dtype=np.float32)
    Wo = np.asarray(Wo, dtype=np.float32)
    bo = np.asarray(bo, dtype=np.float32)

    scale = np.float32(C) ** np.float32(-0.5)
    in_maps = []
    for c in range(N_CORES):
        b, hoff = c // 2, (c % 2) * HC
        heads = slice(hoff, hoff + HC)
        xT_c = np.ascontiguousarray(x[b].T.astype(NPBF16))       # [C, T]
        wq_c = np.ascontiguousarray(
            np.concatenate(list(Wq[heads] * scale), axis=1).astype(NPBF16))
        wk_c = np.ascontiguousarray(
            np.concatenate(list(Wk[heads]), axis=1).astype(NPBF16))
        wv_c = np.ascontiguousarray(
            np.concatenate(list(Wv[heads]), axis=1).astype(NPBF16))
        wot_c = np.ascontiguousarray(
            Wo[:, hoff * D:(hoff + HC) * D].T.astype(NPBF16))    # [512, C]
        bo2_c = np.ascontiguousarray((bo / 2.0).reshape(8, 128).T)       # [128, 8]
        in_maps.append({
            "xT": xT_c, "wq": wq_c, "wk": wk_c, "wv": wv_c,
            "wot": wot_c, "bo2": bo2_c,
            "consts": np.ascontiguousarray(np.concatenate([
                np.eye(128, dtype=np.float32),
                np.triu(np.full((128, 128), -30.0, dtype=np.float32), k=1),
                np.full((128, 128), -30.0, dtype=np.float32),
            ], axis=1).astype(NPBF16)),
        })
    return in_maps


def kernel(x, Wq, Wk, Wv, Wo, bo):
    nc = _get_nc(with_rs=True)
    in_maps = make_in_maps(x, Wq, Wk, Wv, Wo, bo)
    # The axon-tunneled devices occasionally fail transiently
    # (NRT_EXEC_UNIT_UNRECOVERABLE / tunnel hangup); a retry recovers.
    last_err = None
    for _ in range(3):
        try:
            res = run_bass_kernel_spmd(nc, in_maps, list(range(N_CORES))).results
            break
        except Exception as e:  # noqa: BLE001
            last_err = e
            import time
            time.sleep(5)
    else:
        raise last_err

    out = np.empty((B, T, C), dtype=np.float32)
    for c in range(N_CORES):
        b, e = c // 2, c % 2
        yc = np.asarray(res[c]["y"], dtype=np.float32)  # [tt, c' slab, t]
        for tt in range(TTn):
            out[b, tt * 512:(tt + 1) * 512, e * 512:(e + 1) * 512] = yc[tt].T
    return out

